# revision 11
# baseline (speedup 1.0000x reference)
"""Trainium2 Bass kernel for nn_Attention_15418932592994.

GQA attention layer (B=1, S=2048, D=4096, H=32 q-heads, KVH=8 kv-heads,
HD=128) with RoPE + causal mask, tensor-parallel over heads across 8
NeuronCores:

  - each core owns 1 kv-head and its 4 q-heads (column-parallel wq/wk/wv)
  - flash-style attention in "feature-major" layout (no on-device
    transposes except V)
  - AllToAll redistributes attention output from head-sharded to
    sequence-sharded, then every core computes its 256 output rows
    against the full wo (row split of the output instead of an
    all-reduce over partial sums)

Matmuls run in float32r (full PE rate at free-dim >= 256, ~1e-3 rel err).
"""

import sys

import numpy as np

try:
    import concourse.bass as bass  # noqa: F401
except ImportError:
    sys.path.insert(0, "/opt/trn_rl_repo")

import concourse.bass as bass
import concourse.mybir as mybir
import concourse.tile as tile
from concourse import bacc
from concourse.bass_utils import run_bass_kernel_spmd
from concourse.masks import make_identity

F32 = mybir.dt.float32
F32R = mybir.dt.float32r

B, S, D = 1, 2048, 4096
H, KVH, HD = 32, 8, 128
NREP = H // KVH          # 4 q-heads per kv-head
NCORES = 8
HPC = H // NCORES        # 4 q-heads per core
QC = HPC * HD            # 512 q-columns per core
SB = 512                 # seq block for projections / attention sq blocks
NSB = S // SB            # 4
KC = D // 128            # 32 contraction chunks
SKT = S // 128           # 16 key tiles
ROWS = S // NCORES       # 256 output rows per core
SCALE = 1.0 / np.sqrt(HD)
NDBLK = D // SB          # 8 output-dim blocks of 512


def build_program():
    nc = bacc.Bacc("TRN2", target_bir_lowering=False, debug=False,
                   num_devices=NCORES)

    xT = nc.dram_tensor("xT", [D, S], F32R, kind="ExternalInput").ap()
    wq = nc.dram_tensor("wq", [D, QC], F32R, kind="ExternalInput").ap()
    wk = nc.dram_tensor("wk", [D, HD], F32R, kind="ExternalInput").ap()
    wv = nc.dram_tensor("wv", [D, HD], F32R, kind="ExternalInput").ap()
    wo = nc.dram_tensor("wo", [H * HD, D], F32R, kind="ExternalInput").ap()
    cc = nc.dram_tensor("cc", [128, S], F32, kind="ExternalInput").ap()
    ss = nc.dram_tensor("ss", [128, S], F32, kind="ExternalInput").ap()
    maskt = nc.dram_tensor("maskt", [128, NREP * SB], F32,
                           kind="ExternalInput").ap()
    onesv = nc.dram_tensor("onesv", [128, 1], F32R, kind="ExternalInput").ap()
    out = nc.dram_tensor("out", [ROWS, D], F32, kind="ExternalOutput").ap()

    with tile.TileContext(nc) as tc:
        build_tile_kernel(tc, xT, wq, wk, wv, wo, cc, ss, maskt, onesv, out)

    nc.compile()
    return nc


def build_tile_kernel(tc, xT, wq, wk, wv, wo, cc, ss, maskt, onesv, out):
    nc = tc.nc
    import contextlib
    ctx = contextlib.ExitStack()

    # ---------------- pools ----------------
    persist = ctx.enter_context(tc.tile_pool(name="persist", bufs=1))
    dram = ctx.enter_context(tc.tile_pool(name="dram", bufs=1, space="DRAM"))

    # persistent tiles (live through attention)
    qt = [persist.tile([128, S], F32R, tag=f"qt{h}", name=f"qt{h}") for h in range(HPC)]
    kt = persist.tile([128, S], F32R, tag="kt", name="kt")
    vsm = persist.tile([128, S], F32R, tag="vsm", name="vsm")       # 16 [128,128] V tiles
    mt = persist.tile([128, NREP * SB], F32, tag="mt", name="mt")  # diag mask tiles
    ones = persist.tile([128, 1], F32R, tag="ones", name="ones")

    nc.sync.dma_start(out=mt, in_=maskt)
    nc.sync.dma_start(out=ones, in_=onesv)

    a2a_in = dram.tile([H * HD, ROWS], F32R, tag="a2a_in", name="a2a_in")
    a2a_out = dram.tile([H * HD, ROWS], F32R, tag="a2a_out", name="a2a_out")

    # ---------------- phase 1: QKV projections + RoPE + V transpose -------
    with (tc.tile_pool(name="qkvp", bufs=1) as qkvp,
          tc.tile_pool(name="xt_pool", bufs=4) as xt_pool,
          tc.tile_pool(name="rope_pool", bufs=3) as rope_pool,
          tc.tile_pool(name="qkv_psum", bufs=1, space="PSUM") as qkv_psum,
          tc.tile_pool(name="tr_psum", bufs=2, space="PSUM") as tr_psum):
        ident = qkvp.tile([128, 128], F32, tag="ident", name="ident")
        make_identity(nc, ident)

        wq_t = qkvp.tile([128, KC * QC], F32R, tag="wq", name="wq")
        wk_t = qkvp.tile([128, KC * HD], F32R, tag="wk", name="wk")
        wv_t = qkvp.tile([128, KC * HD], F32R, tag="wv", name="wv")
        cc_t = qkvp.tile([128, S], F32, tag="cc", name="cc")
        ss_t = qkvp.tile([128, S], F32, tag="ss", name="ss")
        nc.sync.dma_start(out=cc_t, in_=cc)
        nc.sync.dma_start(out=ss_t, in_=ss)
        for kc in range(KC):
            nc.sync.dma_start(out=wq_t[:, kc * QC:(kc + 1) * QC],
                              in_=wq[kc * 128:(kc + 1) * 128, :])
            nc.sync.dma_start(out=wk_t[:, kc * HD:(kc + 1) * HD],
                              in_=wk[kc * 128:(kc + 1) * 128, :])
            nc.sync.dma_start(out=wv_t[:, kc * HD:(kc + 1) * HD],
                              in_=wv[kc * 128:(kc + 1) * 128, :])

        def rope(dest, src_psum, sb):
            """dest[:, sb*SB:+SB] = rope(src_psum) in even/odd-split layout."""
            sl = slice(sb * SB, (sb + 1) * SB)
            tmp = rope_pool.tile([128, SB], F32, tag="tmp", name="tmp")
            rot = rope_pool.tile([128, SB], F32, tag="rot", name="rot")
            t1 = rope_pool.tile([128, SB], F32, tag="t1", name="t1")
            nc.scalar.copy(tmp, src_psum)
            # partition swap: rot = [odd_half ; even_half]
            nc.sync.dma_start(out=rot[0:64, :], in_=tmp[64:128, :])
            nc.sync.dma_start(out=rot[64:128, :], in_=tmp[0:64, :])
            nc.vector.tensor_mul(t1, tmp, cc_t[:, sl])
            nc.vector.tensor_mul(rot, rot, ss_t[:, sl])  # ss has -sin on top
            nc.vector.tensor_add(dest[:, sl], t1, rot)

        for sb in range(NSB):
            ssl = slice(sb * SB, (sb + 1) * SB)
            xts = []
            for kc in range(KC):
                xt_t = xt_pool.tile([128, SB], F32R, tag="xt", name="xt")
                nc.sync.dma_start(out=xt_t, in_=xT[kc * 128:(kc + 1) * 128, ssl])
                xts.append(xt_t)
            accq = [qkv_psum.tile([128, SB], F32, tag=f"accq{h}", name=f"accq{h}")
                    for h in range(HPC)]
            acck = qkv_psum.tile([128, SB], F32, tag="acck", name="acck")
            accv = qkv_psum.tile([128, SB], F32, tag="accv", name="accv")
            for kc in range(KC):
                st, sp = kc == 0, kc == KC - 1
                for h in range(HPC):
                    nc.tensor.matmul(
                        accq[h],
                        wq_t[:, kc * QC + h * HD: kc * QC + (h + 1) * HD],
                        xts[kc], start=st, stop=sp)
                nc.tensor.matmul(acck, wk_t[:, kc * HD:(kc + 1) * HD],
                                 xts[kc], start=st, stop=sp)
                nc.tensor.matmul(accv, wv_t[:, kc * HD:(kc + 1) * HD],
                                 xts[kc], start=st, stop=sp)
            for h in range(HPC):
                rope(qt[h], accq[h], sb)
            rope(kt, acck, sb)
            # V: copy out of PSUM, then PE-transpose each 128x128 to seq-major
            vt_tmp = rope_pool.tile([128, SB], F32, tag="vt", name="vt")
            nc.scalar.copy(vt_tmp, accv)
            for i in range(SB // 128):
                stile = sb * (SB // 128) + i
                trp = tr_psum.tile([128, 128], F32, tag="trp", name="trp")
                nc.tensor.transpose(trp, vt_tmp[:, i * 128:(i + 1) * 128],
                                    ident)
                nc.scalar.copy(vsm[:, stile * 128:(stile + 1) * 128], trp)

    # ---------------- phase 2: attention (feature-major flash) ------------
    with (tc.tile_pool(name="st_psum", bufs=3, space="PSUM") as st_psum,
          tc.tile_pool(name="ot_psum", bufs=2, space="PSUM") as ot_psum,
          tc.tile_pool(name="z_psum", bufs=2, space="PSUM") as z_psum,
          tc.tile_pool(name="attn", bufs=6) as attn,
          tc.tile_pool(name="norm", bufs=3) as norm):
        for h in range(HPC):
            for j in range(NSB):
                qsl = slice(j * SB, (j + 1) * SB)
                nchunks = NREP * j + NREP
                otp = ot_psum.tile([128, SB], F32, tag="otp", name="otp")
                zp = z_psum.tile([1, SB], F32, tag="zp", name="zp")
                for c in range(nchunks):
                    stp = st_psum.tile([128, SB], F32, tag="stp", name="stp")
                    nc.tensor.matmul(stp, kt[:, c * 128:(c + 1) * 128],
                                     qt[h][:, qsl], start=True, stop=True)
                    sexp = attn.tile([128, SB], F32R, tag="sexp", name="sexp")
                    if c >= NREP * j:  # diagonal chunk: add causal mask
                        t = c - NREP * j
                        msk = attn.tile([128, SB], F32, tag="msk", name="msk")
                        nc.vector.tensor_add(msk, stp,
                                             mt[:, t * SB:(t + 1) * SB])
                        nc.scalar.activation(sexp, msk,
                                             mybir.ActivationFunctionType.Exp,
                                             scale=float(SCALE))
                    else:
                        nc.scalar.activation(sexp, stp,
                                             mybir.ActivationFunctionType.Exp,
                                             scale=float(SCALE))
                    st_, sp_ = c == 0, c == nchunks - 1
                    nc.tensor.matmul(otp, vsm[:, c * 128:(c + 1) * 128],
                                     sexp, start=st_, stop=sp_)
                    nc.tensor.matmul(zp, ones, sexp,
                                     start=st_, stop=sp_)
                zrec = norm.tile([1, SB], F32, tag="zrec", name="zrec")
                nc.vector.reciprocal(zrec, zp)
                zb = norm.tile([128, SB], F32, tag="zb", name="zb")
                nc.gpsimd.partition_broadcast(zb, zrec)
                otn = norm.tile([128, SB], F32R, tag="otn", name="otn")
                nc.vector.tensor_mul(otn, otp, zb)
                # stage into the AllToAll input: seq block j maps to
                # destination cores 2j and 2j+1
                for half in range(2):
                    p = 2 * j + half
                    nc.sync.dma_start(
                        out=a2a_in[p * QC + h * HD:p * QC + (h + 1) * HD, :],
                        in_=otn[:, half * ROWS:(half + 1) * ROWS])

    # ---------------- phase 3: AllToAll (head-shard -> seq-shard) ---------
    nc.gpsimd.collective_compute(
        "AllToAll", mybir.AluOpType.bypass,
        replica_groups=[list(range(NCORES))],
        ins=[a2a_in.opt()], outs=[a2a_out.opt()])

    # ---------------- phase 4: output projection against full wo ----------
    with (tc.tile_pool(name="wop", bufs=1) as wop,
          tc.tile_pool(name="wo_stream", bufs=8) as wo_stream,
          tc.tile_pool(name="wo_psum", bufs=1, space="PSUM") as wo_psum,
          tc.tile_pool(name="bounce", bufs=4) as bounce):
        otg = wop.tile([128, H, ROWS], F32R, tag="otg", name="otg")
        nc.sync.dma_start(
            out=otg, in_=a2a_out.rearrange("(c p) s -> p c s", p=128))
        for pass_ in range(2):
            dofs = pass_ * (D // 2)
            accs = [[wo_psum.tile([128, SB], F32, tag=f"woacc{s_}{d_}", name=f"woacc{s_}{d_}")
                     for d_ in range(NDBLK // 2)] for s_ in range(2)]
            for c in range(H):
                wot = wo_stream.tile([128, D // 2], F32R, tag="wot", name="wot")
                nc.sync.dma_start(
                    out=wot, in_=wo[c * 128:(c + 1) * 128,
                                    dofs:dofs + D // 2])
                st, sp = c == 0, c == H - 1
                for s_ in range(2):
                    lhs = otg[:, c, s_ * 128:(s_ + 1) * 128]
                    for d_ in range(NDBLK // 2):
                        nc.tensor.matmul(
                            accs[s_][d_], lhs,
                            wot[:, d_ * SB:(d_ + 1) * SB],
                            start=st, stop=sp)
            for s_ in range(2):
                for d_ in range(NDBLK // 2):
                    ob = bounce.tile([128, SB], F32, tag="ob", name="ob")
                    nc.vector.tensor_copy(ob, accs[s_][d_])
                    nc.sync.dma_start(
                        out=out[s_ * 128:(s_ + 1) * 128,
                                dofs + d_ * SB:dofs + (d_ + 1) * SB],
                        in_=ob)
    ctx.close()


_PROGRAM = None


def _get_program():
    global _PROGRAM
    if _PROGRAM is None:
        _PROGRAM = build_program()
    return _PROGRAM


def prepare_inputs(x, wq, wk, wv, wo, freqs_cos, freqs_sin, mask):
    """Host-side sharding/layout prep. Returns per-core input maps."""
    x = np.asarray(x, np.float32)
    wq = np.asarray(wq, np.float32)
    wk = np.asarray(wk, np.float32)
    wv = np.asarray(wv, np.float32)
    wo = np.ascontiguousarray(np.asarray(wo, np.float32))
    fc = np.asarray(freqs_cos, np.float32)
    fs = np.asarray(freqs_sin, np.float32)
    mask = np.asarray(mask, np.float32)

    xT = np.ascontiguousarray(x.reshape(S, D).T)
    # even/odd split permutation of each head's 128 columns (RoPE layout)
    perm = np.concatenate([np.arange(0, HD, 2), np.arange(1, HD, 2)])
    wq_h = wq.reshape(D, H, HD)[:, :, perm]
    wk_h = wk.reshape(D, KVH, HD)[:, :, perm]
    wv_h = wv.reshape(D, KVH, HD)

    cosT = fc.T  # [64, S]
    sinT = fs.T
    cc = np.ascontiguousarray(np.concatenate([cosT, cosT], axis=0))
    ss = np.ascontiguousarray(np.concatenate([-sinT, sinT], axis=0))

    m = np.maximum(mask, -1e30)
    mtiles = [np.ascontiguousarray(m[0:SB, t * 128:(t + 1) * 128].T)
              for t in range(NREP)]
    maskt = np.ascontiguousarray(np.concatenate(mtiles, axis=1))

    in_maps = []
    for c in range(NCORES):
        in_maps.append({
            "xT": xT,
            "wq": np.ascontiguousarray(
                wq_h[:, c * HPC:(c + 1) * HPC, :].reshape(D, QC)),
            "wk": np.ascontiguousarray(wk_h[:, c, :]),
            "wv": np.ascontiguousarray(wv_h[:, c, :]),
            "wo": wo,
            "cc": cc,
            "ss": ss,
            "maskt": maskt,
            "onesv": np.ones((128, 1), np.float32),
        })
    return in_maps


def run(in_maps, **kwargs):
    nc = _get_program()
    return run_bass_kernel_spmd(nc, in_maps, core_ids=list(range(NCORES)),
                                **kwargs)


def kernel(x, wq, wk, wv, wo, freqs_cos, freqs_sin, mask, start_pos=0,
           **_ignored):
    in_maps = prepare_inputs(x, wq, wk, wv, wo, freqs_cos, freqs_sin, mask)
    res = run(in_maps)
    full = np.concatenate([res.results[c]["out"] for c in range(NCORES)],
                          axis=0)
    return full.reshape(B, S, D)


if __name__ == "__main__":
    import reference
    inputs = reference.setup_inputs()
    expected = np.asarray(reference.reference(**inputs))
    actual = kernel(**{k: v for k, v in inputs.items()})
    err = np.linalg.norm(actual - expected) / np.linalg.norm(expected)
    print("Relative error:", err)


# revision 12
# speedup vs baseline: 1.1979x; 1.1979x over previous
"""Trainium2 Bass kernel for nn_Attention_15418932592994.

GQA attention layer (B=1, S=2048, D=4096, H=32 q-heads, KVH=8 kv-heads,
HD=128) with RoPE + causal mask, tensor-parallel over heads across 8
NeuronCores:

  - each core owns 1 kv-head and its 4 q-heads (column-parallel wq/wk/wv)
  - flash-style attention in "feature-major" layout (no on-device
    transposes except V)
  - per-head AllToAll redistributes attention output from head-sharded to
    sequence-sharded (overlapped with attention), then every core computes
    its 256 output rows against the full wo (row split of the output
    instead of an all-reduce over partial sums)

Matmul operands are fp16 (inputs are well-scaled; accumulation is fp32 in
PSUM), which halves weight DMA traffic and enables fast weight loads.
"""

import sys

import numpy as np

try:
    import concourse.bass as bass  # noqa: F401
except ImportError:
    sys.path.insert(0, "/opt/trn_rl_repo")

import concourse.bass as bass
import concourse.mybir as mybir
import concourse.tile as tile
from concourse import bacc
from concourse.bass_utils import run_bass_kernel_spmd
from concourse.masks import make_identity

F32 = mybir.dt.float32
F16 = mybir.dt.float16
NPDT = np.float16

B, S, D = 1, 2048, 4096
H, KVH, HD = 32, 8, 128
NREP = H // KVH          # 4 q-heads per kv-head
NCORES = 8
HPC = H // NCORES        # 4 q-heads per core
QC = HPC * HD            # 512 q-columns per core
SB = 512                 # seq block for projections / attention sq blocks
NSB = S // SB            # 4
KC = D // 128            # 32 contraction chunks
ROWS = S // NCORES       # 256 output rows per core
SCALE = 1.0 / np.sqrt(HD)
NDBLK = D // SB          # 8 output-dim blocks of 512


def build_program():
    nc = bacc.Bacc("TRN2", target_bir_lowering=False, debug=False,
                   num_devices=NCORES)

    tensors = dict(
        xT=nc.dram_tensor("xT", [D, S], F16, kind="ExternalInput").ap(),
        wq=nc.dram_tensor("wq", [D, QC], F16, kind="ExternalInput").ap(),
        wk=nc.dram_tensor("wk", [D, HD], F16, kind="ExternalInput").ap(),
        wv=nc.dram_tensor("wv", [D, HD], F16, kind="ExternalInput").ap(),
        wo=nc.dram_tensor("wo", [H * HD, D], F16, kind="ExternalInput").ap(),
        cc=nc.dram_tensor("cc", [128, S], F32, kind="ExternalInput").ap(),
        ss=nc.dram_tensor("ss", [128, S], F32, kind="ExternalInput").ap(),
        maskt=nc.dram_tensor("maskt", [128, NREP * SB], F32,
                             kind="ExternalInput").ap(),
        onesv=nc.dram_tensor("onesv", [128, 1], F16,
                             kind="ExternalInput").ap(),
        out=nc.dram_tensor("out", [ROWS, D], F32, kind="ExternalOutput").ap(),
    )

    with tile.TileContext(nc) as tc:
        build_tile_kernel(tc, **tensors)

    nc.compile()
    return nc


def build_tile_kernel(tc, xT, wq, wk, wv, wo, cc, ss, maskt, onesv, out):
    nc = tc.nc
    import contextlib
    ctx = contextlib.ExitStack()

    persist = ctx.enter_context(tc.tile_pool(name="persist", bufs=1))
    dram = ctx.enter_context(tc.tile_pool(name="dram", bufs=1, space="DRAM"))

    # persistent tiles (live through attention)
    qt = [persist.tile([128, S], F16, tag=f"qt{h}", name=f"qt{h}")
          for h in range(HPC)]
    kt = persist.tile([128, S], F16, tag="kt", name="kt")
    vsm = persist.tile([128, S], F16, tag="vsm", name="vsm")
    mt = persist.tile([128, NREP * SB], F32, tag="mt", name="mt")
    ones = persist.tile([128, 1], F16, tag="ones", name="ones")

    nc.sync.dma_start(out=mt, in_=maskt)
    nc.sync.dma_start(out=ones, in_=onesv)

    # per-head AllToAll buffers: [8 dest cores x 128 rows, 256 cols]
    a2a_in = [dram.tile([NCORES * HD, ROWS], F16, tag=f"a2a_in{h}",
                        name=f"a2a_in{h}") for h in range(HPC)]
    a2a_out = [dram.tile([NCORES * HD, ROWS], F16, tag=f"a2a_out{h}",
                         name=f"a2a_out{h}") for h in range(HPC)]

    # ---------------- phase 1: QKV projections + RoPE + V transpose -------
    with (tc.tile_pool(name="qkvp", bufs=1) as qkvp,
          tc.tile_pool(name="xt_pool", bufs=4) as xt_pool,
          tc.tile_pool(name="rope_pool", bufs=3) as rope_pool,
          tc.tile_pool(name="qkv_psum", bufs=1, space="PSUM") as qkv_psum,
          tc.tile_pool(name="tr_psum", bufs=2, space="PSUM") as tr_psum):
        ident = qkvp.tile([128, 128], F16, tag="ident", name="ident")
        make_identity(nc, ident)

        wq_t = qkvp.tile([128, KC * QC], F16, tag="wq", name="wq")
        wk_t = qkvp.tile([128, KC * HD], F16, tag="wk", name="wk")
        wv_t = qkvp.tile([128, KC * HD], F16, tag="wv", name="wv")
        cc_t = qkvp.tile([128, S], F32, tag="cc", name="cc")
        ss_t = qkvp.tile([128, S], F32, tag="ss", name="ss")
        nc.sync.dma_start(out=cc_t, in_=cc)
        nc.sync.dma_start(out=ss_t, in_=ss)

        def load_w(kc):
            nc.sync.dma_start(out=wq_t[:, kc * QC:(kc + 1) * QC],
                              in_=wq[kc * 128:(kc + 1) * 128, :])
            nc.sync.dma_start(out=wk_t[:, kc * HD:(kc + 1) * HD],
                              in_=wk[kc * 128:(kc + 1) * 128, :])
            nc.sync.dma_start(out=wv_t[:, kc * HD:(kc + 1) * HD],
                              in_=wv[kc * 128:(kc + 1) * 128, :])

        def rope(dest, src_psum, sb):
            """dest[:, sb*SB:+SB] = rope(src_psum) in even/odd-split layout."""
            sl = slice(sb * SB, (sb + 1) * SB)
            tmp = rope_pool.tile([128, SB], F32, tag="tmp", name="tmp")
            rot = rope_pool.tile([128, SB], F32, tag="rot", name="rot")
            t1 = rope_pool.tile([128, SB], F32, tag="t1", name="t1")
            nc.scalar.copy(tmp, src_psum)
            # partition swap: rot = [odd_half ; even_half]
            nc.sync.dma_start(out=rot[0:64, :], in_=tmp[64:128, :])
            nc.sync.dma_start(out=rot[64:128, :], in_=tmp[0:64, :])
            nc.vector.tensor_mul(t1, tmp, cc_t[:, sl])
            nc.vector.tensor_mul(rot, rot, ss_t[:, sl])  # ss has -sin on top
            nc.vector.tensor_add(dest[:, sl], t1, rot)

        for sb in range(NSB):
            ssl = slice(sb * SB, (sb + 1) * SB)
            xts = []
            for kc in range(KC):
                if sb == 0:
                    load_w(kc)  # interleave weight loads with first block
                xt_t = xt_pool.tile([128, SB], F16, tag="xt", name="xt")
                nc.sync.dma_start(out=xt_t,
                                  in_=xT[kc * 128:(kc + 1) * 128, ssl])
                xts.append(xt_t)
            accq = [qkv_psum.tile([128, SB], F32, tag=f"accq{h}",
                                  name=f"accq{h}") for h in range(HPC)]
            acck = qkv_psum.tile([128, SB], F32, tag="acck", name="acck")
            accv = qkv_psum.tile([128, SB], F32, tag="accv", name="accv")
            for kc in range(KC):
                st, sp = kc == 0, kc == KC - 1
                for h in range(HPC):
                    nc.tensor.matmul(
                        accq[h],
                        wq_t[:, kc * QC + h * HD: kc * QC + (h + 1) * HD],
                        xts[kc], start=st, stop=sp)
                nc.tensor.matmul(acck, wk_t[:, kc * HD:(kc + 1) * HD],
                                 xts[kc], start=st, stop=sp)
                nc.tensor.matmul(accv, wv_t[:, kc * HD:(kc + 1) * HD],
                                 xts[kc], start=st, stop=sp)
            for h in range(HPC):
                rope(qt[h], accq[h], sb)
            rope(kt, acck, sb)
            # V: copy out of PSUM, then PE-transpose each 128x128 to seq-major
            vt_tmp = rope_pool.tile([128, SB], F16, tag="vt", name="vt")
            nc.scalar.copy(vt_tmp, accv)
            for i in range(SB // 128):
                stile = sb * (SB // 128) + i
                trp = tr_psum.tile([128, 128], F16, tag="trp", name="trp")
                nc.tensor.transpose(trp, vt_tmp[:, i * 128:(i + 1) * 128],
                                    ident)
                nc.scalar.copy(vsm[:, stile * 128:(stile + 1) * 128], trp)

    # ---------------- phase 2: attention + per-head AllToAll --------------
    # wo prefetch pool opened alongside attention so its DMAs can run early
    wo_stream = ctx.enter_context(tc.tile_pool(name="wo_stream", bufs=16))
    wo_tiles = {}
    for pass_ in range(2):
        for c in range(H):
            wot = wo_stream.tile([128, D // 2], F16, tag="wot",
                                 name=f"wot{pass_}_{c}")
            nc.sync.dma_start(
                out=wot,
                in_=wo[c * 128:(c + 1) * 128,
                       pass_ * (D // 2):(pass_ + 1) * (D // 2)])
            wo_tiles[(pass_, c)] = wot

    with (tc.tile_pool(name="st_psum", bufs=3, space="PSUM") as st_psum,
          tc.tile_pool(name="ot_psum", bufs=2, space="PSUM") as ot_psum,
          tc.tile_pool(name="z_psum", bufs=2, space="PSUM") as z_psum,
          tc.tile_pool(name="attn", bufs=6) as attn,
          tc.tile_pool(name="norm", bufs=3) as norm):
        for h in range(HPC):
            for j in range(NSB):
                qsl = slice(j * SB, (j + 1) * SB)
                nchunks = NREP * j + NREP
                otp = ot_psum.tile([128, SB], F32, tag="otp", name="otp")
                zp = z_psum.tile([1, SB], F32, tag="zp", name="zp")
                for c in range(nchunks):
                    stp = st_psum.tile([128, SB], F32, tag="stp", name="stp")
                    nc.tensor.matmul(stp, kt[:, c * 128:(c + 1) * 128],
                                     qt[h][:, qsl], start=True, stop=True)
                    sexp = attn.tile([128, SB], F16, tag="sexp", name="sexp")
                    if c >= NREP * j:  # diagonal chunk: add causal mask
                        t = c - NREP * j
                        msk = attn.tile([128, SB], F32, tag="msk", name="msk")
                        nc.vector.tensor_add(msk, stp,
                                             mt[:, t * SB:(t + 1) * SB])
                        nc.scalar.activation(sexp, msk,
                                             mybir.ActivationFunctionType.Exp,
                                             scale=float(SCALE))
                    else:
                        nc.scalar.activation(sexp, stp,
                                             mybir.ActivationFunctionType.Exp,
                                             scale=float(SCALE))
                    st_, sp_ = c == 0, c == nchunks - 1
                    nc.tensor.matmul(otp, vsm[:, c * 128:(c + 1) * 128],
                                     sexp, start=st_, stop=sp_)
                    nc.tensor.matmul(zp, ones, sexp, start=st_, stop=sp_)
                # normalize: broadcast Z to 128 partitions, then reciprocal
                zrow = norm.tile([1, SB], F32, tag="zrow", name="zrow")
                nc.scalar.copy(zrow, zp)
                zbr = norm.tile([128, SB], F32, tag="zbr", name="zbr")
                nc.gpsimd.partition_broadcast(zbr, zrow)
                zb = norm.tile([128, SB], F32, tag="zb", name="zb")
                nc.vector.reciprocal(zb, zbr)
                otn = norm.tile([128, SB], F16, tag="otn", name="otn")
                nc.vector.tensor_mul(otn, otp, zb)
                # stage into head-h AllToAll input: seq block j -> cores
                # 2j and 2j+1
                for half in range(2):
                    p = 2 * j + half
                    nc.sync.dma_start(
                        out=a2a_in[h][p * HD:(p + 1) * HD, :],
                        in_=otn[:, half * ROWS:(half + 1) * ROWS])
            # head h fully staged on every core (SPMD) -> exchange it now
            nc.gpsimd.collective_compute(
                "AllToAll", mybir.AluOpType.bypass,
                replica_groups=[list(range(NCORES))],
                ins=[a2a_in[h].opt()], outs=[a2a_out[h].opt()])

    # ---------------- phase 4: output projection against full wo ----------
    with (tc.tile_pool(name="wop", bufs=1) as wop,
          tc.tile_pool(name="wo_psum", bufs=1, space="PSUM") as wo_psum,
          tc.tile_pool(name="bounce", bufs=4) as bounce):
        # gather [4096, 256] -> SBUF, chunk-major: global head g = 4p + h
        otg = wop.tile([128, H, ROWS], F16, tag="otg", name="otg")
        for h in range(HPC):
            # a2a_out[h] rows p*128..(p+1)*128 = core p's head h at my cols
            nc.sync.dma_start(
                out=otg.rearrange("q (p hh) s -> q p hh s", hh=HPC)[:, :, h, :],
                in_=a2a_out[h].rearrange("(p q) s -> q p s", q=128))
        for pass_ in range(2):
            dofs = pass_ * (D // 2)
            accs = [[wo_psum.tile([128, SB], F32, tag=f"woacc{s_}{d_}",
                                  name=f"woacc{s_}{d_}")
                     for d_ in range(NDBLK // 2)] for s_ in range(2)]
            for c in range(H):
                wot = wo_tiles[(pass_, c)]
                st, sp = c == 0, c == H - 1
                for s_ in range(2):
                    lhs = otg[:, c, s_ * 128:(s_ + 1) * 128]
                    for d_ in range(NDBLK // 2):
                        nc.tensor.matmul(
                            accs[s_][d_], lhs,
                            wot[:, d_ * SB:(d_ + 1) * SB],
                            start=st, stop=sp)
            for s_ in range(2):
                for d_ in range(NDBLK // 2):
                    ob = bounce.tile([128, SB], F32, tag="ob", name="ob")
                    nc.vector.tensor_copy(ob, accs[s_][d_])
                    nc.sync.dma_start(
                        out=out[s_ * 128:(s_ + 1) * 128,
                                dofs + d_ * SB:dofs + (d_ + 1) * SB],
                        in_=ob)
    ctx.close()


_PROGRAM = None


def _get_program():
    global _PROGRAM
    if _PROGRAM is None:
        _PROGRAM = build_program()
    return _PROGRAM


def prepare_inputs(x, wq, wk, wv, wo, freqs_cos, freqs_sin, mask):
    """Host-side sharding/layout prep. Returns per-core input maps."""
    x = np.asarray(x, np.float32)
    wq = np.asarray(wq, np.float32)
    wk = np.asarray(wk, np.float32)
    wv = np.asarray(wv, np.float32)
    wo = np.ascontiguousarray(np.asarray(wo, np.float32).astype(NPDT))
    fc = np.asarray(freqs_cos, np.float32)
    fs = np.asarray(freqs_sin, np.float32)
    mask = np.asarray(mask, np.float32)

    xT = np.ascontiguousarray(x.reshape(S, D).T.astype(NPDT))
    # even/odd split permutation of each head's 128 columns (RoPE layout)
    perm = np.concatenate([np.arange(0, HD, 2), np.arange(1, HD, 2)])
    wq_h = wq.reshape(D, H, HD)[:, :, perm].astype(NPDT)
    wk_h = wk.reshape(D, KVH, HD)[:, :, perm].astype(NPDT)
    wv_h = wv.reshape(D, KVH, HD).astype(NPDT)

    cosT = fc.T  # [64, S]
    sinT = fs.T
    cc = np.ascontiguousarray(np.concatenate([cosT, cosT], axis=0))
    ss = np.ascontiguousarray(np.concatenate([-sinT, sinT], axis=0))

    m = np.maximum(mask, -1e30)
    mtiles = [np.ascontiguousarray(m[0:SB, t * 128:(t + 1) * 128].T)
              for t in range(NREP)]
    maskt = np.ascontiguousarray(np.concatenate(mtiles, axis=1))

    in_maps = []
    for c in range(NCORES):
        in_maps.append({
            "xT": xT,
            "wq": np.ascontiguousarray(
                wq_h[:, c * HPC:(c + 1) * HPC, :].reshape(D, QC)),
            "wk": np.ascontiguousarray(wk_h[:, c, :]),
            "wv": np.ascontiguousarray(wv_h[:, c, :]),
            "wo": wo,
            "cc": cc,
            "ss": ss,
            "maskt": maskt,
            "onesv": np.ones((128, 1), NPDT),
        })
    return in_maps


def run(in_maps, **kwargs):
    nc = _get_program()
    return run_bass_kernel_spmd(nc, in_maps, core_ids=list(range(NCORES)),
                                **kwargs)


def kernel(x, wq, wk, wv, wo, freqs_cos, freqs_sin, mask, start_pos=0,
           **_ignored):
    in_maps = prepare_inputs(x, wq, wk, wv, wo, freqs_cos, freqs_sin, mask)
    res = run(in_maps)
    full = np.concatenate([res.results[c]["out"] for c in range(NCORES)],
                          axis=0)
    return full.reshape(B, S, D)


if __name__ == "__main__":
    import reference
    inputs = reference.setup_inputs()
    expected = np.asarray(reference.reference(**inputs))
    actual = kernel(**{k: v for k, v in inputs.items()})
    err = np.linalg.norm(actual - expected) / np.linalg.norm(expected)
    print("Relative error:", err)


# revision 14
# speedup vs baseline: 1.3510x; 1.1278x over previous
"""Trainium2 Bass kernel for nn_Attention_15418932592994.

GQA attention layer (B=1, S=2048, D=4096, H=32 q-heads, KVH=8 kv-heads,
HD=128) with RoPE + causal mask, tensor-parallel over heads across 8
NeuronCores:

  - each core owns 1 kv-head and its 4 q-heads (column-parallel wq/wk/wv)
  - flash-style attention in "feature-major" layout (no on-device
    transposes except V)
  - per-head AllToAll redistributes attention output from head-sharded to
    sequence-sharded (overlapped with attention), then every core computes
    its 256 output rows against the full wo (row split of the output
    instead of an all-reduce over partial sums)

Matmul operands are fp16 (inputs are well-scaled; accumulation is fp32 in
PSUM), which halves weight DMA traffic and enables fast weight loads.
"""

import sys

import numpy as np

try:
    import concourse.bass as bass  # noqa: F401
except ImportError:
    sys.path.insert(0, "/opt/trn_rl_repo")

import concourse.bass as bass
import concourse.mybir as mybir
import concourse.tile as tile
from concourse import bacc
from concourse.bass_utils import run_bass_kernel_spmd
from concourse.masks import make_identity

F32 = mybir.dt.float32
F16 = mybir.dt.float16
NPDT = np.float16

B, S, D = 1, 2048, 4096
H, KVH, HD = 32, 8, 128
NREP = H // KVH          # 4 q-heads per kv-head
NCORES = 8
HPC = H // NCORES        # 4 q-heads per core
QC = HPC * HD            # 512 q-columns per core
SB = 512                 # seq block for projections / attention sq blocks
NSB = S // SB            # 4
KC = D // 128            # 32 contraction chunks
ROWS = S // NCORES       # 256 output rows per core
SCALE = 1.0 / np.sqrt(HD)
NDBLK = D // SB          # 8 output-dim blocks of 512
KG = 8                   # contraction chunks per DMA batch


def build_program():
    nc = bacc.Bacc("TRN2", target_bir_lowering=False, debug=False,
                   num_devices=NCORES)

    tensors = dict(
        xT=nc.dram_tensor("xT", [D, S], F16, kind="ExternalInput").ap(),
        wq=nc.dram_tensor("wq", [D, QC], F16, kind="ExternalInput").ap(),
        wk=nc.dram_tensor("wk", [D, HD], F16, kind="ExternalInput").ap(),
        wv=nc.dram_tensor("wv", [D, HD], F16, kind="ExternalInput").ap(),
        wo=nc.dram_tensor("wo", [H * HD, D], F16, kind="ExternalInput").ap(),
        cc=nc.dram_tensor("cc", [128, S], F32, kind="ExternalInput").ap(),
        ss=nc.dram_tensor("ss", [128, S], F32, kind="ExternalInput").ap(),
        maskt=nc.dram_tensor("maskt", [128, NREP * SB], F32,
                             kind="ExternalInput").ap(),
        onesv=nc.dram_tensor("onesv", [128, 1], F16,
                             kind="ExternalInput").ap(),
        out=nc.dram_tensor("out", [ROWS, D], F32, kind="ExternalOutput").ap(),
    )

    with tile.TileContext(nc) as tc:
        build_tile_kernel(tc, **tensors)

    nc.compile()
    return nc


def build_tile_kernel(tc, xT, wq, wk, wv, wo, cc, ss, maskt, onesv, out):
    nc = tc.nc
    import contextlib
    ctx = contextlib.ExitStack()

    persist = ctx.enter_context(tc.tile_pool(name="persist", bufs=1))
    dram = ctx.enter_context(tc.tile_pool(name="dram", bufs=1, space="DRAM"))

    # persistent tiles (live through attention)
    qt = [persist.tile([128, S], F16, tag=f"qt{h}", name=f"qt{h}")
          for h in range(HPC)]
    kt = persist.tile([128, S], F16, tag="kt", name="kt")
    vsm = persist.tile([128, S], F16, tag="vsm", name="vsm")
    mt = persist.tile([128, NREP * SB], F32, tag="mt", name="mt")
    ones = persist.tile([128, 1], F16, tag="ones", name="ones")

    # per-head AllToAll buffers: [8 dest cores x 128 rows, 256 cols]
    a2a_in = [dram.tile([NCORES * HD, ROWS], F16, tag=f"a2a_in{h}",
                        name=f"a2a_in{h}") for h in range(HPC)]
    a2a_out = [dram.tile([NCORES * HD, ROWS], F16, tag=f"a2a_out{h}",
                         name=f"a2a_out{h}") for h in range(HPC)]

    # ---------------- phase 1: QKV projections + RoPE + V transpose -------
    with (tc.tile_pool(name="qkvp", bufs=1) as qkvp,
          tc.tile_pool(name="xt_pool", bufs=2) as xt_pool,
          tc.tile_pool(name="rope_pool", bufs=3) as rope_pool,
          tc.tile_pool(name="qkv_psum", bufs=1, space="PSUM") as qkv_psum,
          tc.tile_pool(name="tr_psum", bufs=2, space="PSUM") as tr_psum):
        wq_t = qkvp.tile([128, KC * QC], F16, tag="wq", name="wq")
        wk_t = qkvp.tile([128, KC * HD], F16, tag="wk", name="wk")
        wv_t = qkvp.tile([128, KC * HD], F16, tag="wv", name="wv")
        cc_t = qkvp.tile([128, S], F32, tag="cc", name="cc")
        ss_t = qkvp.tile([128, S], F32, tag="ss", name="ss")
        ident = qkvp.tile([128, 128], F16, tag="ident", name="ident")

        # batched weight loads: one strided DMA per group of KG k-chunks
        wqr = wq.rearrange("(kc p) c -> p kc c", p=128)
        wq_tr = wq_t.rearrange("p (kc c) -> p kc c", c=QC)
        wkr = wk.rearrange("(kc p) c -> p kc c", p=128)
        wk_tr = wk_t.rearrange("p (kc c) -> p kc c", c=HD)
        wvr = wv.rearrange("(kc p) c -> p kc c", p=128)
        wv_tr = wv_t.rearrange("p (kc c) -> p kc c", c=HD)
        xtr = xT.rearrange("(kc p) s -> p kc s", p=128)

        def load_wgroup(g):
            gs = slice(g * KG, (g + 1) * KG)
            nc.sync.dma_start(out=wq_tr[:, gs, :], in_=wqr[:, gs, :])
            nc.sync.dma_start(out=wk_tr[:, gs, :], in_=wkr[:, gs, :])
            nc.sync.dma_start(out=wv_tr[:, gs, :], in_=wvr[:, gs, :])

        def rope(dest, src_psum, sb):
            """dest[:, sb*SB:+SB] = rope(src_psum) in even/odd-split layout."""
            sl = slice(sb * SB, (sb + 1) * SB)
            tmp = rope_pool.tile([128, SB], F32, tag="tmp", name="tmp")
            rot = rope_pool.tile([128, SB], F32, tag="rot", name="rot")
            t1 = rope_pool.tile([128, SB], F32, tag="t1", name="t1")
            nc.scalar.copy(tmp, src_psum)
            # partition swap: rot = [odd_half ; even_half]
            nc.gpsimd.dma_start(out=rot[0:64, :], in_=tmp[64:128, :])
            nc.gpsimd.dma_start(out=rot[64:128, :], in_=tmp[0:64, :])
            nc.vector.tensor_mul(t1, tmp, cc_t[:, sl])
            nc.vector.tensor_mul(rot, rot, ss_t[:, sl])  # ss has -sin on top
            nc.vector.tensor_add(dest[:, sl], t1, rot)

        for sb in range(NSB):
            ssl = slice(sb * SB, (sb + 1) * SB)
            xts = xt_pool.tile([128, KC, SB], F16, tag="xt", name="xt")
            for g in range(KC // KG):
                if sb == 0:
                    load_wgroup(g)
                gs = slice(g * KG, (g + 1) * KG)
                nc.sync.dma_start(out=xts[:, gs, :], in_=xtr[:, gs, ssl])
            if sb == 0:
                # deferred so they don't gate the first matmuls
                nc.sync.dma_start(out=cc_t, in_=cc)
                nc.sync.dma_start(out=ss_t, in_=ss)
                make_identity(nc, ident)
            accq = [qkv_psum.tile([128, SB], F32, tag=f"accq{h}",
                                  name=f"accq{h}") for h in range(HPC)]
            acck = qkv_psum.tile([128, SB], F32, tag="acck", name="acck")
            accv = qkv_psum.tile([128, SB], F32, tag="accv", name="accv")
            for kc in range(KC):
                st, sp = kc == 0, kc == KC - 1
                for h in range(HPC):
                    nc.tensor.matmul(
                        accq[h],
                        wq_t[:, kc * QC + h * HD: kc * QC + (h + 1) * HD],
                        xts[:, kc, :], start=st, stop=sp)
                nc.tensor.matmul(acck, wk_t[:, kc * HD:(kc + 1) * HD],
                                 xts[:, kc, :], start=st, stop=sp)
                nc.tensor.matmul(accv, wv_t[:, kc * HD:(kc + 1) * HD],
                                 xts[:, kc, :], start=st, stop=sp)
            for h in range(HPC):
                rope(qt[h], accq[h], sb)
            rope(kt, acck, sb)
            # V: copy out of PSUM, then PE-transpose each 128x128 to seq-major
            vt_tmp = rope_pool.tile([128, SB], F16, tag="vt", name="vt")
            nc.scalar.copy(vt_tmp, accv)
            for i in range(SB // 128):
                stile = sb * (SB // 128) + i
                trp = tr_psum.tile([128, 128], F16, tag="trp", name="trp")
                nc.tensor.transpose(trp, vt_tmp[:, i * 128:(i + 1) * 128],
                                    ident)
                nc.scalar.copy(vsm[:, stile * 128:(stile + 1) * 128], trp)

    nc.sync.dma_start(out=mt, in_=maskt)
    nc.sync.dma_start(out=ones, in_=onesv)

    # ---------------- phase 2: attention + per-head AllToAll --------------
    # wo prefetch issued on the (otherwise idle) gpsimd queue so it streams
    # in during attention
    wo_stream = ctx.enter_context(tc.tile_pool(name="wo_stream", bufs=16))
    wo_tiles = {}
    for pass_ in range(2):
        for c in range(H):
            wot = wo_stream.tile([128, D // 2], F16, tag="wot",
                                 name=f"wot{pass_}_{c}")
            nc.gpsimd.dma_start(
                out=wot,
                in_=wo[c * 128:(c + 1) * 128,
                       pass_ * (D // 2):(pass_ + 1) * (D // 2)])
            wo_tiles[(pass_, c)] = wot

    with (tc.tile_pool(name="st_psum", bufs=3, space="PSUM") as st_psum,
          tc.tile_pool(name="ot_psum", bufs=2, space="PSUM") as ot_psum,
          tc.tile_pool(name="z_psum", bufs=2, space="PSUM") as z_psum,
          tc.tile_pool(name="attn", bufs=8) as attn,
          tc.tile_pool(name="norm", bufs=3) as norm,
          tc.tile_pool(name="stage", bufs=6) as stage):
        for h in range(HPC):
            for j in range(NSB):
                qsl = slice(j * SB, (j + 1) * SB)
                nchunks = NREP * j + NREP
                otp = ot_psum.tile([128, SB], F32, tag="otp", name="otp")
                zp = z_psum.tile([1, SB], F32, tag="zp", name="zp")
                for c in range(nchunks):
                    stp = st_psum.tile([128, SB], F32, tag="stp", name="stp")
                    nc.tensor.matmul(stp, kt[:, c * 128:(c + 1) * 128],
                                     qt[h][:, qsl], start=True, stop=True)
                    sexp = attn.tile([128, SB], F16, tag="sexp", name="sexp")
                    if c >= NREP * j:  # diagonal chunk: add causal mask
                        t = c - NREP * j
                        msk = attn.tile([128, SB], F32, tag="msk", name="msk")
                        nc.vector.tensor_add(msk, stp,
                                             mt[:, t * SB:(t + 1) * SB])
                        nc.scalar.activation(sexp, msk,
                                             mybir.ActivationFunctionType.Exp,
                                             scale=float(SCALE))
                    else:
                        nc.scalar.activation(sexp, stp,
                                             mybir.ActivationFunctionType.Exp,
                                             scale=float(SCALE))
                    st_, sp_ = c == 0, c == nchunks - 1
                    nc.tensor.matmul(otp, vsm[:, c * 128:(c + 1) * 128],
                                     sexp, start=st_, stop=sp_)
                    nc.tensor.matmul(zp, ones, sexp, start=st_, stop=sp_)
                # normalize: broadcast Z to 128 partitions, fast reciprocal
                zrow = norm.tile([1, SB], F32, tag="zrow", name="zrow")
                nc.scalar.copy(zrow, zp)
                zbr = norm.tile([128, SB], F32, tag="zbr", name="zbr")
                nc.gpsimd.partition_broadcast(zbr, zrow)
                zb = norm.tile([128, SB], F32, tag="zb", name="zb")
                nc.vector.reciprocal_approx_fast(out=zb, in_=zbr)
                otn = stage.tile([128, SB], F16, tag="otn", name="otn")
                nc.vector.tensor_mul(otn, otp, zb)
                # stage into head-h AllToAll input: seq block j -> cores
                # 2j and 2j+1
                for half in range(2):
                    p = 2 * j + half
                    nc.sync.dma_start(
                        out=a2a_in[h][p * HD:(p + 1) * HD, :],
                        in_=otn[:, half * ROWS:(half + 1) * ROWS])
            # head h fully staged on every core (SPMD) -> exchange it now
            nc.gpsimd.collective_compute(
                "AllToAll", mybir.AluOpType.bypass,
                replica_groups=[list(range(NCORES))],
                ins=[a2a_in[h].opt()], outs=[a2a_out[h].opt()])

    # ---------------- phase 4: output projection against full wo ----------
    with (tc.tile_pool(name="wop", bufs=1) as wop,
          tc.tile_pool(name="wo_psum", bufs=1, space="PSUM") as wo_psum,
          tc.tile_pool(name="bounce", bufs=4) as bounce):
        # gather [4096, 256] -> SBUF, chunk-major: global head g = 4p + h
        otg = wop.tile([128, H, ROWS], F16, tag="otg", name="otg")
        for h in range(HPC):
            # a2a_out[h] rows p*128..(p+1)*128 = core p's head h at my cols
            nc.sync.dma_start(
                out=otg.rearrange("q (p hh) s -> q p hh s", hh=HPC)[:, :, h, :],
                in_=a2a_out[h].rearrange("(p q) s -> q p s", q=128))
        for pass_ in range(2):
            dofs = pass_ * (D // 2)
            accs = [[wo_psum.tile([128, SB], F32, tag=f"woacc{s_}{d_}",
                                  name=f"woacc{s_}{d_}")
                     for d_ in range(NDBLK // 2)] for s_ in range(2)]
            for c in range(H):
                wot = wo_tiles[(pass_, c)]
                st, sp = c == 0, c == H - 1
                for s_ in range(2):
                    lhs = otg[:, c, s_ * 128:(s_ + 1) * 128]
                    for d_ in range(NDBLK // 2):
                        nc.tensor.matmul(
                            accs[s_][d_], lhs,
                            wot[:, d_ * SB:(d_ + 1) * SB],
                            start=st, stop=sp)
            for s_ in range(2):
                for d_ in range(NDBLK // 2):
                    ob = bounce.tile([128, SB], F32, tag="ob", name="ob")
                    nc.vector.tensor_copy(ob, accs[s_][d_])
                    nc.sync.dma_start(
                        out=out[s_ * 128:(s_ + 1) * 128,
                                dofs + d_ * SB:dofs + (d_ + 1) * SB],
                        in_=ob)
    ctx.close()


_PROGRAM = None


def _get_program():
    global _PROGRAM
    if _PROGRAM is None:
        _PROGRAM = build_program()
    return _PROGRAM


def prepare_inputs(x, wq, wk, wv, wo, freqs_cos, freqs_sin, mask):
    """Host-side sharding/layout prep. Returns per-core input maps."""
    x = np.asarray(x, np.float32)
    wq = np.asarray(wq, np.float32)
    wk = np.asarray(wk, np.float32)
    wv = np.asarray(wv, np.float32)
    wo = np.ascontiguousarray(np.asarray(wo, np.float32).astype(NPDT))
    fc = np.asarray(freqs_cos, np.float32)
    fs = np.asarray(freqs_sin, np.float32)
    mask = np.asarray(mask, np.float32)

    xT = np.ascontiguousarray(x.reshape(S, D).T.astype(NPDT))
    # even/odd split permutation of each head's 128 columns (RoPE layout)
    perm = np.concatenate([np.arange(0, HD, 2), np.arange(1, HD, 2)])
    wq_h = wq.reshape(D, H, HD)[:, :, perm].astype(NPDT)
    wk_h = wk.reshape(D, KVH, HD)[:, :, perm].astype(NPDT)
    wv_h = wv.reshape(D, KVH, HD).astype(NPDT)

    cosT = fc.T  # [64, S]
    sinT = fs.T
    cc = np.ascontiguousarray(np.concatenate([cosT, cosT], axis=0))
    ss = np.ascontiguousarray(np.concatenate([-sinT, sinT], axis=0))

    m = np.maximum(mask, -1e30)
    mtiles = [np.ascontiguousarray(m[0:SB, t * 128:(t + 1) * 128].T)
              for t in range(NREP)]
    maskt = np.ascontiguousarray(np.concatenate(mtiles, axis=1))

    in_maps = []
    for c in range(NCORES):
        in_maps.append({
            "xT": xT,
            "wq": np.ascontiguousarray(
                wq_h[:, c * HPC:(c + 1) * HPC, :].reshape(D, QC)),
            "wk": np.ascontiguousarray(wk_h[:, c, :]),
            "wv": np.ascontiguousarray(wv_h[:, c, :]),
            "wo": wo,
            "cc": cc,
            "ss": ss,
            "maskt": maskt,
            "onesv": np.ones((128, 1), NPDT),
        })
    return in_maps


def run(in_maps, **kwargs):
    nc = _get_program()
    return run_bass_kernel_spmd(nc, in_maps, core_ids=list(range(NCORES)),
                                **kwargs)


def kernel(x, wq, wk, wv, wo, freqs_cos, freqs_sin, mask, start_pos=0,
           **_ignored):
    in_maps = prepare_inputs(x, wq, wk, wv, wo, freqs_cos, freqs_sin, mask)
    res = run(in_maps)
    full = np.concatenate([res.results[c]["out"] for c in range(NCORES)],
                          axis=0)
    return full.reshape(B, S, D)


if __name__ == "__main__":
    import reference
    inputs = reference.setup_inputs()
    expected = np.asarray(reference.reference(**inputs))
    actual = kernel(**{k: v for k, v in inputs.items()})
    err = np.linalg.norm(actual - expected) / np.linalg.norm(expected)
    print("Relative error:", err)
